# revision 18
# baseline (speedup 1.0000x reference)
"""Multi-head attention (B=8, L=1024, D=1024, H=16, dk=dv=64) on 8 trn2 cores.

Sharding: data-parallel over batch; core b computes batch element b fully.
No collectives. Host pre-transposes weights/activations and un-transposes the
attention-probability output (device writes it [h, k, q]).

Matmuls run in float32r (fp32 read-truncated to fp22, full PE rate). The
fp32r self-loading matmul ISA instruction has a single semaphore-wait slot,
so the kernel is structured so every fp32r matmul needs at most one fresh
wait: x/w projection operands share one DMA, small constants are produced on
ScalarE (same lane as the PSUM-copy producers), and each accumulation group
opens with a zero bf16 matmul (lowered to LDW+MM = two wait slots) that
absorbs the PSUM-slot-release wait.
"""

import sys

sys.path.insert(0, "/opt/trn_rl_repo")

import numpy as np
import ml_dtypes

import concourse.bass as bass
from concourse import bacc
import concourse.mybir as mybir
import concourse.tile as tile
from concourse.bass_utils import run_bass_kernel_spmd

B, L, D = 8, 1024, 1024
H, DKV = 16, 64
P = 128
NT = 8  # 128-tiles along L or D
FD = 512  # matmul moving free-dim chunk
QC = 2  # chunks of FD along L
NEG = -1e10
EPS = 1e-5

f32 = mybir.dt.float32
f32r = mybir.dt.float32r
bf16 = mybir.dt.bfloat16
AF = mybir.ActivationFunctionType

_CACHED_NC = None


def build_nc():
    nc = bacc.Bacc("TRN2", num_devices=B)

    xw_q = nc.dram_tensor("xw_q", (2, D, L), f32r, kind="ExternalInput").ap()
    xw_k = nc.dram_tensor("xw_k", (2, D, L), f32r, kind="ExternalInput").ap()
    xw_v = nc.dram_tensor("xw_v", (2, D, L), f32r, kind="ExternalInput").ap()
    qres = nc.dram_tensor("qres", (L, D), f32, kind="ExternalInput").ap()
    maskT = nc.dram_tensor("maskT", (L, L), bf16, kind="ExternalInput").ap()
    wfcT = nc.dram_tensor("wfcT", (D, D), f32, kind="ExternalInput").ap()
    lnvecs = nc.dram_tensor("lnvecs", (1, 3 * D), f32, kind="ExternalInput").ap()

    attn_t = nc.dram_tensor("attn_t", (H, L, L), f32, kind="ExternalOutput").ap()
    yout = nc.dram_tensor("yout", (L, D), f32, kind="ExternalOutput").ap()

    c1_1p = nc.const_aps.tensor(1.0, (1, 1))   # [1,1] const
    c1_p1 = nc.const_aps.tensor(1.0, (P, 1))   # [128,1] const

    with tile.TileContext(nc) as tc:
        with tc.tile_pool(name="poolA", bufs=1) as poolA:
            # ---- constants ----
            negI = poolA.tile([P, P], bf16, tag="negI", name="negI")
            nc.gpsimd.memset(negI, 0.0)
            nc.gpsimd.affine_select(
                out=negI,
                in_=negI,
                compare_op=mybir.AluOpType.not_equal,
                fill=NEG,
                base=0,
                pattern=[[-1, P]],
                channel_multiplier=1,
            )
            # ScalarE-produced constants (ACT lane merges with PSUM-copy waits)
            ones1 = poolA.tile([1, P], f32r, tag="ones1", name="ones1")
            nc.scalar.copy(ones1, c1_1p.to_broadcast((1, P)))
            zbf = poolA.tile([1, FD], bf16, tag="zbf", name="zbf")
            nc.scalar.activation(
                zbf, c1_1p.to_broadcast((1, FD)), AF.Identity, bias=0.0, scale=0.0
            )

            def opener(psum_ap, m):
                # zero bf16 matmul opening an accumulation group; carries the
                # PSUM-release wait (bf16 lowers to LDW+MM: two wait slots).
                nc.tensor.matmul(
                    psum_ap,
                    zbf[0:1, 0:m],
                    zbf,
                    start=True,
                    stop=False,
                )

            # persistent activations
            poolD = tc.alloc_tile_pool(name="poolD", bufs=1)
            ot_sb = poolD.tile([P, NT, L], f32r, tag="otsb", name="ot_sb")

            poolH = tc.alloc_tile_pool(name="poolH", bufs=1)
            qhT = poolH.tile([P, NT, L], f32r, tag="qhT", name="qhT")
            khT = poolH.tile([P, NT, L], f32r, tag="khT", name="khT")

            pool_b1 = tc.alloc_tile_pool(name="poolB1", bufs=1)
            vaug = pool_b1.tile([P, NT, H, DKV + 1], f32r, tag="vaug", name="vaug")

            # ================= stage 1: Q/K projections =================
            with (
                tc.tile_pool(name="poolC", bufs=1) as poolC,
                tc.tile_pool(name="ps12", bufs=2, space="PSUM") as ps12,
            ):
                for nm, xw_d, dest in (("q", xw_q, qhT), ("k", xw_k, khT)):
                    xw = poolC.tile([P, 2, NT, L], f32r, tag="xw", name=f"xw_{nm}")
                    nc.sync.dma_start(
                        xw, xw_d.rearrange("b (dt p) l -> p b dt l", p=P)
                    )
                    for ot in range(NT):
                        for qc in range(QC):
                            ps = ps12.tile(
                                [P, FD], f32, tag="proj", name=f"ps_{nm}_{ot}_{qc}"
                            )
                            opener(ps, P)
                            for dt in range(NT):
                                nc.tensor.matmul(
                                    ps,
                                    xw[:, 1, dt, ot * P : (ot + 1) * P],
                                    xw[:, 0, dt, qc * FD : (qc + 1) * FD],
                                    start=False,
                                    stop=(dt == NT - 1),
                                )
                            nc.scalar.copy(dest[:, ot, qc * FD : (qc + 1) * FD], ps)

                # ============= stage 2: V projection (natural, +ones col) =====
                xwv = poolC.tile([P, 2, NT, L], f32r, tag="xw", name="xw_v_sb")
                nc.sync.dma_start(
                    xwv, xw_v.rearrange("b (dt p) l -> p b dt l", p=P)
                )
                # ones column via ScalarE (reads const, x*0+1)
                nc.scalar.activation(
                    vaug[:, :, :, DKV : DKV + 1],
                    c1_p1.to_broadcast((P, NT, H, 1)),
                    AF.Identity,
                    bias=1.0,
                    scale=0.0,
                )
                for lt in range(NT):
                    for oc in range(QC):
                        ps = ps12.tile([P, FD], f32, tag="proj", name=f"ps_v_{lt}_{oc}")
                        opener(ps, P)
                        for dt in range(NT):
                            nc.tensor.matmul(
                                ps,
                                xwv[:, 0, dt, lt * P : (lt + 1) * P],
                                xwv[:, 1, dt, oc * FD : (oc + 1) * FD],
                                start=False,
                                stop=(dt == NT - 1),
                            )
                        nc.scalar.copy(
                            vaug[:, lt, oc * 8 : (oc + 1) * 8, 0:DKV],
                            ps.rearrange("p (h d) -> p h d", d=DKV),
                        )

            # ================= stage 3: attention per head-pair ============
            pool_b2 = tc.alloc_tile_pool(name="poolB2", bufs=1)
            maskTs = pool_b2.tile([P, NT, L], bf16, tag="maskTs", name="maskTs")
            nc.sync.dma_start(maskTs, maskT.rearrange("(kt p) q -> p kt q", p=P))
            with (
                tc.tile_pool(name="poolPT", bufs=18) as poolPT,
                tc.tile_pool(name="poolW3", bufs=2) as poolW3,
                tc.tile_pool(name="poolOut", bufs=4) as poolOut,
                tc.tile_pool(name="ps_st", bufs=1, space="PSUM") as ps_st,
                tc.tile_pool(name="ps_pv", bufs=2, space="PSUM") as ps_pv,
                tc.tile_pool(name="ps_rep", bufs=2, space="PSUM") as ps_rep,
            ):
                # warm PE's gpsimd clock past the negI producer
                warm = ps_rep.tile([P, FD], f32, tag="rep", name="warm")
                nc.tensor.matmul(warm[:, 0:P], negI, negI, start=True, stop=True)

                for hp in range(H // 2):
                    for qc in range(QC):
                        qsl = slice(qc * FD, (qc + 1) * FD)
                        pvps = []
                        for hh in range(2):
                            h = 2 * hp + hh
                            pv = ps_pv.tile(
                                [DKV + 1, FD], f32, tag=f"pv{hh}", name=f"pv_{h}_{qc}"
                            )
                            opener(pv, DKV + 1)
                            pvps.append(pv)
                        pts = {}
                        for kt in range(NT):
                            stps = []
                            for hh in range(2):
                                h = 2 * hp + hh
                                po = hh * DKV
                                stp = ps_st.tile(
                                    [P, FD],
                                    f32,
                                    tag=f"st{hh}",
                                    name=f"st_{h}_{qc}_{kt}",
                                )
                                nc.tensor.matmul(
                                    stp,
                                    khT[po : po + DKV, hp, kt * P : (kt + 1) * P],
                                    qhT[po : po + DKV, hp, qsl],
                                    start=True,
                                    stop=False,
                                )
                                stps.append(stp)
                            for hh in range(2):
                                nc.tensor.matmul(
                                    stps[hh],
                                    negI,
                                    maskTs[:, kt, qsl],
                                    start=False,
                                    stop=True,
                                )
                            for hh in range(2):
                                h = 2 * hp + hh
                                pt = poolPT.tile(
                                    [P, FD], f32r, tag="pt", name=f"pt_{h}_{qc}_{kt}"
                                )
                                nc.scalar.activation(pt, stps[hh], AF.Exp, scale=0.125)
                                nc.tensor.matmul(
                                    pvps[hh],
                                    vaug[:, kt, h, :],
                                    pt,
                                    start=False,
                                    stop=(kt == NT - 1),
                                )
                                pts[(hh, kt)] = pt
                        for hh in range(2):
                            h = 2 * hp + hh
                            po = hh * DKV
                            # row-sums -> replicate -> reciprocal
                            sumrow = poolW3.tile(
                                [1, FD], f32r, tag="sumrow", name=f"sum_{h}_{qc}"
                            )
                            nc.scalar.copy(sumrow, pvps[hh][DKV : DKV + 1, :])
                            rep = ps_rep.tile(
                                [P, FD], f32, tag="rep", name=f"rep_{h}_{qc}"
                            )
                            opener(rep, P)
                            nc.tensor.matmul(rep, ones1, sumrow, start=False, stop=True)
                            rsb = poolW3.tile(
                                [P, FD], f32, tag="rsb", name=f"r_{h}_{qc}", bufs=4
                            )
                            nc.vector.reciprocal(rsb, rep)
                            # normalize O^T block in PSUM, then copy (ACT only)
                            nc.vector.tensor_mul(
                                pvps[hh][0:DKV, :], pvps[hh][0:DKV, :], rsb[0:DKV, :]
                            )
                            nc.scalar.copy(
                                ot_sb[po : po + DKV, hp, qsl], pvps[hh][0:DKV, :]
                            )
                            # normalize P^T and write out
                            for kt in range(NT):
                                po_t = poolOut.tile(
                                    [P, FD], f32, tag="pout", name=f"po_{h}_{qc}_{kt}"
                                )
                                nc.vector.tensor_mul(
                                    po_t, pts[(hh, kt)].bitcast(f32), rsb
                                )
                                nc.sync.dma_start(
                                    attn_t[h, kt * P : (kt + 1) * P, qsl], po_t
                                )

            pool_b2.release()
            pool_b1.release()
            poolH.release()

            # ================= stage 5: FC + residual + LayerNorm ========
            with (
                tc.tile_pool(name="poolE", bufs=1) as poolE,
                tc.tile_pool(name="poolW5", bufs=2) as poolW5,
                tc.tile_pool(name="ps5", bufs=2, space="PSUM") as ps5,
            ):
                wfc_raw = poolE.tile([P, NT, D], f32, tag="wfc_raw", name="wfc_raw")
                wfc = poolE.tile([P, NT, D], f32r, tag="wfc", name="wfc")
                nc.sync.dma_start(
                    wfc_raw, wfcT.rearrange("(ot p) d -> p ot d", p=P)
                )
                for ot in range(NT):
                    nc.scalar.copy(wfc[:, ot], wfc_raw[:, ot])
                lnv_raw = poolE.tile([1, 3 * D], f32, tag="lnv_raw", name="lnv_raw")
                nc.sync.dma_start(lnv_raw, lnvecs)
                lnv = poolE.tile([1, 3 * D], f32r, tag="lnv", name="lnv")
                nc.scalar.copy(lnv, lnv_raw)
                reps = []
                for i in range(3):
                    rp = poolE.tile([P, D], f32, tag=f"lnrep{i}", name=f"lnrep{i}")
                    for c in range(QC):
                        rps = ps5.tile([P, FD], f32, tag="vrep", name=f"lnps_{i}_{c}")
                        opener(rps, P)
                        nc.tensor.matmul(
                            rps,
                            ones1,
                            lnv[0:1, i * D + c * FD : i * D + (c + 1) * FD],
                            start=False,
                            stop=True,
                        )
                        nc.scalar.copy(rp[:, c * FD : (c + 1) * FD], rps)
                    reps.append(rp)
                fcb_rep, g_rep, b_rep = reps
                epsb = poolE.tile([P, 1], f32, tag="epsb", name="epsb")
                nc.vector.memset(epsb, EPS)

                for lt in range(NT):
                    res = poolW5.tile([P, D], f32, tag="res", name=f"res_{lt}")
                    nc.sync.dma_start(res, qres[lt * P : (lt + 1) * P, :])
                    x = poolW5.tile([P, D], f32, tag="xln", name=f"x_{lt}")
                    for dc in range(QC):
                        dsl = slice(dc * FD, (dc + 1) * FD)
                        fps = ps5.tile([P, FD], f32, tag="fc", name=f"fc_{lt}_{dc}")
                        opener(fps, P)
                        for ot in range(NT):
                            nc.tensor.matmul(
                                fps,
                                ot_sb[:, ot, lt * P : (lt + 1) * P],
                                wfc[:, ot, dsl],
                                start=False,
                                stop=(ot == NT - 1),
                            )
                        nc.vector.tensor_add(x[:, dsl], fps, res[:, dsl])
                        nc.vector.tensor_add(x[:, dsl], x[:, dsl], fcb_rep[:, dsl])
                    # LayerNorm over free dim
                    srow = poolW5.tile([P, 1], f32, tag="srow", name=f"srow_{lt}")
                    nc.vector.tensor_reduce(
                        srow, x, mybir.AxisListType.X, mybir.AluOpType.add
                    )
                    negmu = poolW5.tile([P, 1], f32, tag="negmu", name=f"negmu_{lt}")
                    nc.vector.tensor_scalar_mul(negmu, srow, -1.0 / D)
                    xc = poolW5.tile([P, D], f32, tag="xc", name=f"xc_{lt}")
                    nc.vector.tensor_scalar_add(xc, x, negmu)
                    ssq = poolW5.tile([P, 1], f32, tag="ssq", name=f"ssq_{lt}")
                    nc.scalar.activation(x, xc, AF.Square, accum_out=ssq)
                    std = poolW5.tile([P, 1], f32, tag="std", name=f"std_{lt}")
                    nc.scalar.activation(std, ssq, AF.Sqrt, bias=epsb, scale=1.0 / D)
                    rstd = poolW5.tile([P, 1], f32, tag="rstd", name=f"rstd_{lt}")
                    nc.vector.reciprocal(rstd, std)
                    yt = poolW5.tile([P, D], f32, tag="yt", name=f"yt_{lt}", bufs=3)
                    nc.vector.tensor_scalar_mul(yt, xc, rstd)
                    nc.vector.tensor_mul(yt, yt, g_rep)
                    nc.vector.tensor_add(yt, yt, b_rep)
                    nc.sync.dma_start(yout[lt * P : (lt + 1) * P, :], yt)

            poolD.release()

    nc.compile()
    return nc


def _get_nc():
    global _CACHED_NC
    if _CACHED_NC is None:
        _CACHED_NC = build_nc()
    return _CACHED_NC


LAST_RESULTS = None


def kernel(q, k, v, mask, w_qs, w_ks, w_vs, fc_w, fc_b, ln_g, ln_b, trace=False):
    global LAST_RESULTS
    q = np.asarray(q, dtype=np.float32)
    k = np.asarray(k, dtype=np.float32)
    v = np.asarray(v, dtype=np.float32)
    mask = np.asarray(mask)
    wqT = np.asarray(w_qs, dtype=np.float32).T
    wkT = np.asarray(w_ks, dtype=np.float32).T
    wvT = np.asarray(w_vs, dtype=np.float32).T
    wfcT = np.ascontiguousarray(np.asarray(fc_w, dtype=np.float32).T)
    lnvecs = np.concatenate(
        [
            np.asarray(fc_b, dtype=np.float32),
            np.asarray(ln_g, dtype=np.float32),
            np.asarray(ln_b, dtype=np.float32),
        ]
    ).reshape(1, 3 * D)

    in_maps = []
    for b in range(B):
        in_maps.append(
            {
                "xw_q": np.ascontiguousarray(np.stack([q[b].T, wqT])),
                "xw_k": np.ascontiguousarray(np.stack([k[b].T, wkT])),
                "xw_v": np.ascontiguousarray(np.stack([v[b].T, wvT])),
                "qres": np.ascontiguousarray(q[b]),
                "maskT": np.ascontiguousarray(mask[b].T).astype(ml_dtypes.bfloat16),
                "wfcT": wfcT,
                "lnvecs": lnvecs,
            }
        )

    nc = _get_nc()
    res = run_bass_kernel_spmd(nc, in_maps, core_ids=list(range(B)), trace=trace)
    LAST_RESULTS = res

    y_full = np.empty((B, L, D), dtype=np.float32)
    attn_full = np.empty((H * B, L, L), dtype=np.float32)
    for b in range(B):
        out = res.results[b]
        y_full[b] = out["yout"]
        at = out["attn_t"]  # [H, k, q]
        for h in range(H):
            attn_full[h * B + b] = at[h].T
    return y_full, attn_full
